# revision 1
# baseline (speedup 1.0000x reference)
"""DLSA block (clustered sparse attention) Trainium2 kernel.

Full-input contract: kernel(**inputs) takes the complete unsharded tensors,
shards batch-dim across 8 NeuronCores, runs a Bass/Tile kernel per core, and
gathers the full output on host.

Host-side marshaling: h_geo/h_pos are uploaded pre-transposed per cluster
([B, C, D, S] layout) so the kernel needs no on-chip transposes and DMA
descriptors are 512B (cluster-feature rows) instead of 128B point rows.

Algebraic folds done on host (weight-space only, float64 for accuracy):
  A    = Wq^T @ Wk / sqrt(D)      -> scores S = Xg A Xg^T + (bq Wk/sqrt(D)) Xg^T
  bk drops entirely (adds a per-row constant to scores; softmax-invariant).
  Wvo  = Wo @ Wv                  -> V' = Xp Wvo^T  (V and O projections fused)
  bo2  = bo + Wo @ bv             (bv commutes through attention since rows of
                                   softmax sum to 1; added to V' pre-attention)

Per cluster (S=128 pts, D=32 feats) on device:
  Z'^T[f,s] = blockdiag(A)^T Xg^T + c   (one matmul per 4-cluster group)
  S^T[t,s]  = Xg Z'^T             (4 row-banded matmuls, one PSUM bank/band)
  P^T       = exp(S^T)            (one ACT op per group)
  V''[t,g]  = Xp blockdiag(Wvo)^T + bo2 (one matmul + one batched bias-add)
  F[s,g]    = P^T.T @ [V''|1]     (ones col yields softmax denom r in col 32)
  out       = F * (1/r)           (batched strided evac into the store tile)
"""

import sys

for _p in ("/opt/trn_rl_repo",):
    if _p not in sys.path:
        sys.path.insert(0, _p)

from contextlib import ExitStack

import numpy as np

import concourse.bass as bass
import concourse.tile as tile
from concourse import bacc, mybir
from concourse.bass_utils import run_bass_kernel_spmd

F32 = mybir.dt.float32

B, N, D = 16, 16384, 32
C_TOTAL, S = 128, 128          # clusters per batch, points per cluster
N_CORES = 8
B_LOC = B // N_CORES           # batches per core
ROWS = B_LOC * N               # data rows per core
TROWS = B_LOC * C_TOTAL * D    # rows of the transposed layout [(b,c,f), s]
SC_CLUSTERS = 32               # clusters per superchunk
SC_ROWS = SC_CLUSTERS * S      # output rows per superchunk
SC_TROWS = SC_CLUSTERS * D     # transposed rows per superchunk
N_SC = ROWS // SC_ROWS         # 8 superchunks per core
G = 4                          # clusters per group
GROUPS_PER_SC = SC_CLUSTERS // G


def _build_program():
    nc = bacc.Bacc("TRN2", target_bir_lowering=False, debug=False)

    hgT = nc.dram_tensor("hgT", [TROWS, S], F32, kind="ExternalInput").ap()
    hpT = nc.dram_tensor("hpT", [TROWS, S], F32, kind="ExternalInput").ap()
    a_blk = nc.dram_tensor("a_blk", [128, 128], F32, kind="ExternalInput").ap()
    cvec = nc.dram_tensor("cvec", [128, 1], F32, kind="ExternalInput").ap()
    wvo_blk = nc.dram_tensor("wvo_blk", [128, 128], F32, kind="ExternalInput").ap()
    bo2_rep = nc.dram_tensor("bo2_rep", [128, G * D], F32, kind="ExternalInput").ap()
    out = nc.dram_tensor("out", [ROWS, D], F32, kind="ExternalOutput").ap()

    with tile.TileContext(nc) as tc, ExitStack() as ctx:
        consts = ctx.enter_context(tc.tile_pool(name="consts", bufs=1))
        io_pool = ctx.enter_context(tc.tile_pool(name="io", bufs=2))
        zsb_pool = ctx.enter_context(tc.tile_pool(name="zsb", bufs=2))
        p_pool = ctx.enter_context(tc.tile_pool(name="p", bufs=2))
        small_pool = ctx.enter_context(tc.tile_pool(name="small", bufs=4))
        v33_pool = ctx.enter_context(tc.tile_pool(name="v33", bufs=1))

        # PSUM: 8 banks. Row-band-concurrent matmuls must land in distinct
        # banks per band (same-partition same-bank concurrent drains from
        # different sub-array row bands wedge the device).
        ps_z = ctx.enter_context(tc.tile_pool(name="ps_z", bufs=1, space="PSUM"))
        ps_work = ctx.enter_context(tc.tile_pool(name="ps_work", bufs=1, space="PSUM"))
        ps_v = ctx.enter_context(tc.tile_pool(name="ps_v", bufs=1, space="PSUM"))
        ps_f = ctx.enter_context(tc.tile_pool(name="ps_f", bufs=2, space="PSUM"))

        # constants
        a_sb = consts.tile([128, 128], F32, tag="a_sb")
        nc.sync.dma_start(a_sb[:], a_blk)
        cvec_sb = consts.tile([128, 1], F32, tag="cvec_sb")
        nc.sync.dma_start(cvec_sb[:], cvec)
        wvo_sb = consts.tile([128, 128], F32, tag="wvo_sb")
        nc.sync.dma_start(wvo_sb[:], wvo_blk)
        bo2_sb = consts.tile([128, G * D], F32, tag="bo2_sb")
        nc.sync.dma_start(bo2_sb[:], bo2_rep)

        # v33 ring: [t, (c,33)] with ones in col 32 of each 33-block
        v33_tiles = []
        for i in range(4):
            t = v33_pool.tile([128, G * 33], F32, tag=f"v33_{i}")
            ones_ap = t[:].rearrange("p (c g) -> p c g", g=33)[:, :, 32:33]
            nc.vector.memset(ones_ap, 1.0)
            v33_tiles.append(t)

        g_global = 0
        for sc in range(N_SC):
            rows = slice(sc * SC_ROWS, (sc + 1) * SC_ROWS)
            trow0 = sc * SC_TROWS
            # hgT/hpT superchunk: [(c4,f)=128, (j, s)] — group j's block-diag
            # transposed inputs land directly in matmul-operand layout.
            # Loads split in half so group 0 can start early.
            hg_sc = io_pool.tile([128, GROUPS_PER_SC * S], F32, tag="hg_sc")
            hp_sc = io_pool.tile([128, GROUPS_PER_SC * S], F32, tag="hp_sc")
            q_j = GROUPS_PER_SC // 4
            for h in range(4):
                r0 = trow0 + h * q_j * 128
                jcols = slice(h * q_j * S, (h + 1) * q_j * S)
                nc.sync.dma_start(
                    hg_sc[:, jcols].rearrange("p (j s) -> p j s", j=q_j),
                    hgT[r0 : r0 + q_j * 128, :].rearrange(
                        "(j r) s -> r j s", j=q_j
                    ),
                )
                nc.sync.dma_start(
                    hp_sc[:, jcols].rearrange("p (j s) -> p j s", j=q_j),
                    hpT[r0 : r0 + q_j * 128, :].rearrange(
                        "(j r) s -> r j s", j=q_j
                    ),
                )
            out_sc = io_pool.tile([128, SC_CLUSTERS * D], F32, tag="out_sc")

            for j in range(GROUPS_PER_SC):
                cols = slice(j * G * D, (j + 1) * G * D)
                xg = hg_sc[:, j * S : (j + 1) * S]
                xp = hp_sc[:, j * S : (j + 1) * S]

                # Z'^T[(c,f),s] = blockdiag(A)^T Xg^T (+c at evac)
                z_ps = ps_z.tile([128, 128], F32, tag="z_ps")
                nc.tensor.matmul(z_ps[:], a_sb[:], xg)
                z_sb = zsb_pool.tile([128, 128], F32, tag="z_sb")
                nc.scalar.activation(
                    z_sb[:], z_ps[:], mybir.ActivationFunctionType.Identity,
                    bias=cvec_sb[:],
                )

                # S^T[t,s] = Xg Z'^T: 4 row-banded matmuls, one bank per band
                wk = ps_work.tile([128, 2048], F32, tag="wk")
                for c in range(G):
                    p0 = c * 32
                    nc.tensor.matmul(
                        wk[:, c * 512 : c * 512 + 128],
                        xg[p0 : p0 + 32, :],
                        z_sb[p0 : p0 + 32, :],
                        tile_position=(p0, 0),
                    )
                wk_view = wk[:].rearrange("p (c q) -> p c q", q=512)
                p_sb = p_pool.tile([128, 512], F32, tag="p_sb")
                nc.scalar.activation(
                    p_sb[:].rearrange("p (c q) -> p c q", q=128),
                    wk_view[:, :, 0:128],
                    mybir.ActivationFunctionType.Exp,
                )

                # V'[t,(c,g)] = Xp blockdiag(Wvo^T): one matmul
                v_ps = ps_v.tile([128, 128], F32, tag="v_ps")
                nc.tensor.matmul(v_ps[:], xp, wvo_sb[:])
                # V'' = V' + bo2, strided into the v33 ring (ones col kept)
                v33 = v33_tiles[g_global % 4]
                nc.vector.tensor_tensor(
                    v33[:].rearrange("p (c g) -> p c g", g=33)[:, :, 0:32],
                    v_ps[:].rearrange("p (c g) -> p c g", g=D),
                    bo2_sb[:].rearrange("p (c g) -> p c g", g=D),
                    mybir.AluOpType.add,
                )

                # F_un[s,(c,33)] = P^T.T @ [V''|1]; col 32 of block = r[s]
                f_ps = ps_f.tile([128, G * 33], F32, tag="f_ps")
                for c in range(G):
                    nc.tensor.matmul(
                        f_ps[:, c * 33 : (c + 1) * 33],
                        p_sb[:, c * 128 : (c + 1) * 128],
                        v33[:, c * 33 : (c + 1) * 33],
                        tile_position=(0, 0),
                    )
                f_view = f_ps[:].rearrange("p (c g) -> p c g", g=33)
                recip = small_pool.tile([128, G], F32, tag="recip")
                nc.vector.reciprocal(recip[:, :, None], f_view[:, :, 32:33])
                nc.vector.tensor_tensor(
                    out_sc[:, cols].rearrange("p (c d) -> p c d", d=D),
                    f_view[:, :, 0:32],
                    recip[:, :, None].to_broadcast([128, G, D]),
                    mybir.AluOpType.mult,
                )
                g_global += 1

            # store in halves so the first half drains while the second half
            # of the superchunk is still computing
            hc = SC_CLUSTERS // 2
            for h in range(2):
                hrows = slice(
                    sc * SC_ROWS + h * hc * S, sc * SC_ROWS + (h + 1) * hc * S
                )
                hcols = slice(h * hc * D, (h + 1) * hc * D)
                nc.sync.dma_start(
                    out[hrows, :].rearrange("(c s) d -> s c d", s=S),
                    out_sc[:, hcols].rearrange("p (c d) -> p c d", d=D),
                )

    nc.compile()
    return nc


_PROGRAM = None


def _get_program():
    global _PROGRAM
    if _PROGRAM is None:
        _PROGRAM = _build_program()
    return _PROGRAM


def _host_fold(Wq, bq, Wk, bk, Wv, bv, Wo, bo):
    Wq64, Wk64 = np.asarray(Wq, np.float64), np.asarray(Wk, np.float64)
    Wv64, Wo64 = np.asarray(Wv, np.float64), np.asarray(Wo, np.float64)
    bq64, bv64, bo64 = (np.asarray(x, np.float64) for x in (bq, bv, bo))
    scale = 1.0 / np.sqrt(np.float64(D))
    A = (Wq64.T @ Wk64) * scale                      # [e, f]
    c = (bq64 @ Wk64) * scale                        # [f]
    WvoT = (Wo64 @ Wv64).T                           # [e, g]
    bo2 = bo64 + Wo64 @ bv64                         # [g]
    a_blk = np.zeros((128, 128), np.float32)
    wvo_blk = np.zeros((128, 128), np.float32)
    for cc in range(G):
        a_blk[cc * D : (cc + 1) * D, cc * D : (cc + 1) * D] = A
        wvo_blk[cc * D : (cc + 1) * D, cc * D : (cc + 1) * D] = WvoT
    cvec = np.tile(c, G)[:, None].astype(np.float32)         # [128, 1]
    bo2_rep = np.tile(bo2, (128, G)).reshape(128, G * D).astype(np.float32)
    return a_blk, cvec, wvo_blk, bo2_rep


def make_in_maps(h_pos, h_geo, Wq, bq, Wk, bk, Wv, bv, Wo, bo):
    a_blk, cvec, wvo_blk, bo2_rep = _host_fold(Wq, bq, Wk, bk, Wv, bv, Wo, bo)
    # per-cluster transpose on host: [B, N, D] -> [B, C, D, S]
    hgT_full = np.ascontiguousarray(
        np.asarray(h_geo, np.float32).reshape(B, C_TOTAL, S, D).transpose(0, 1, 3, 2)
    ).reshape(B * C_TOTAL * D, S)
    hpT_full = np.ascontiguousarray(
        np.asarray(h_pos, np.float32).reshape(B, C_TOTAL, S, D).transpose(0, 1, 3, 2)
    ).reshape(B * C_TOTAL * D, S)
    in_maps = []
    for core in range(N_CORES):
        trows = slice(core * TROWS, (core + 1) * TROWS)
        in_maps.append(
            {
                "hgT": np.ascontiguousarray(hgT_full[trows]),
                "hpT": np.ascontiguousarray(hpT_full[trows]),
                "a_blk": a_blk,
                "cvec": cvec,
                "wvo_blk": wvo_blk,
                "bo2_rep": bo2_rep,
            }
        )
    return in_maps


def kernel(h_pos, h_geo, n_clusters, Wq, bq, Wk, bk, Wv, bv, Wo, bo, **kwargs):
    assert int(n_clusters) == C_TOTAL
    nc = _get_program()
    in_maps = make_in_maps(h_pos, h_geo, Wq, bq, Wk, bk, Wv, bv, Wo, bo)
    res = run_bass_kernel_spmd(nc, in_maps, core_ids=list(range(N_CORES)))
    shards = [r["out"].reshape(B_LOC, N, D) for r in res.results]
    return np.concatenate(shards, axis=0).astype(np.float32)



# revision 3
# speedup vs baseline: 2.2086x; 2.2086x over previous
"""DLSA block (clustered sparse attention) Trainium2 kernel.

Full-input contract: kernel(**inputs) takes the complete unsharded tensors,
shards batch-dim across 8 NeuronCores, runs a Bass/Tile kernel per core, and
gathers the full output on host.

v2: bf16 datapath. All matmul operands are bf16 (fp32 PSUM accumulate,
fp32 output store); host-measured pipeline rel err ~6e-3. bf16 matmuls run
at 1 cycle/row on the PE array vs 4 for fp32, and halve input DMA bytes.

Host-side marshaling: h_geo/h_pos are packed per core into superchunk slabs
that match the SBUF layout exactly ([sc, (c4 f)=128, (j s)=1024]), so every
DMA descriptor is a contiguous 2KB partition row. The output is stored as
[s=128, (sc j c d)=8192] and unpacked on host (4KB descriptors).

Algebraic folds done on host (weight-space only, float64 for accuracy):
  A    = Wq^T @ Wk / sqrt(D)      -> scores S = Xg A Xg^T + (bq Wk/sqrt(D)) Xg^T
  bk drops entirely (adds a per-row constant to scores; softmax-invariant).
  Wvo  = Wo @ Wv                  -> V' = Xp Wvo^T  (V and O projections fused)
  bo2  = bo + Wo @ bv             (bv commutes through attention since rows of
                                   softmax sum to 1; added to V' pre-attention)

Per superchunk (32 clusters of S=128 pts, D=32 feats) on device:
  Z'^T slab = blockdiag(A)^T Xg^T (+c)  2 wide matmuls (N=512) + 2 ACT evacs
  per group j of 4 clusters:
    S^T[t,s]  = Xg Z'^T            4 row-banded matmuls, one PSUM bank/band
    P^T       = exp(S^T) -> bf16   one ACT op per group
    V'[t,g]   = Xp blockdiag(Wvo)^T  one matmul (N=128)
    V''       = V' + bo2 -> bf16   vector add, strided into the v33 ring
    F[s,g]    = P^T.T @ [V''|1]    ones col yields softmax denom r in col 32
  per group pair: out = F * (1/r)  one reciprocal + one batched multiply
"""

import sys

for _p in ("/opt/trn_rl_repo",):
    if _p not in sys.path:
        sys.path.insert(0, _p)

from contextlib import ExitStack

import numpy as np
import ml_dtypes

import concourse.bass as bass
import concourse.tile as tile
from concourse import bacc, mybir
from concourse.bass_utils import run_bass_kernel_spmd

F32 = mybir.dt.float32
BF16 = mybir.dt.bfloat16
NP_BF16 = ml_dtypes.bfloat16

B, N, D = 16, 16384, 32
C_TOTAL, S = 128, 128          # clusters per batch, points per cluster
N_CORES = 8
B_LOC = B // N_CORES           # batches per core
ROWS = B_LOC * N               # data rows per core
SC_CLUSTERS = 32               # clusters per superchunk
SC_ROWS = SC_CLUSTERS * S      # output rows per superchunk
N_SC = ROWS // SC_ROWS         # 8 superchunks per core
G = 4                          # clusters per group
GROUPS_PER_SC = SC_CLUSTERS // G
SLAB_COLS = GROUPS_PER_SC * S  # 1024 cols per input slab


def _build_program():
    nc = bacc.Bacc("TRN2", target_bir_lowering=False, debug=False)

    hg_p = nc.dram_tensor("hg_p", [N_SC * 128, SLAB_COLS], BF16,
                          kind="ExternalInput").ap()
    hp_p = nc.dram_tensor("hp_p", [N_SC * 128, SLAB_COLS], BF16,
                          kind="ExternalInput").ap()
    a_blk = nc.dram_tensor("a_blk", [128, 128], BF16, kind="ExternalInput").ap()
    cvec = nc.dram_tensor("cvec", [128, 1], F32, kind="ExternalInput").ap()
    wvo_blk = nc.dram_tensor("wvo_blk", [128, 128], BF16,
                             kind="ExternalInput").ap()
    bo2_rep = nc.dram_tensor("bo2_rep", [128, G * D], F32,
                             kind="ExternalInput").ap()
    out = nc.dram_tensor("out", [128, N_SC * SC_CLUSTERS * D], F32,
                         kind="ExternalOutput").ap()

    with tile.TileContext(nc) as tc, ExitStack() as ctx:
        consts = ctx.enter_context(tc.tile_pool(name="consts", bufs=1))
        io_pool = ctx.enter_context(tc.tile_pool(name="io", bufs=2))
        zsb_pool = ctx.enter_context(tc.tile_pool(name="zsb", bufs=2))
        p_pool = ctx.enter_context(tc.tile_pool(name="p", bufs=2))
        small_pool = ctx.enter_context(tc.tile_pool(name="small", bufs=4))
        v33_pool = ctx.enter_context(tc.tile_pool(name="v33", bufs=1))

        # PSUM is 8 banks of 2KB/partition, bank-granular allocation:
        #   z 1 + wk 4 (one bank per score band) + v 1 + f 2 = 8 banks
        ps_z = ctx.enter_context(tc.tile_pool(name="ps_z", bufs=1, space="PSUM"))
        ps_work = ctx.enter_context(tc.tile_pool(name="ps_work", bufs=1, space="PSUM"))
        ps_v = ctx.enter_context(tc.tile_pool(name="ps_v", bufs=1, space="PSUM"))
        ps_f = ctx.enter_context(tc.tile_pool(name="ps_f", bufs=2, space="PSUM"))

        # constants
        a_sb = consts.tile([128, 128], BF16, tag="a_sb")
        nc.sync.dma_start(a_sb[:], a_blk)
        cvec_sb = consts.tile([128, 1], F32, tag="cvec_sb")
        nc.sync.dma_start(cvec_sb[:], cvec)
        wvo_sb = consts.tile([128, 128], BF16, tag="wvo_sb")
        nc.sync.dma_start(wvo_sb[:], wvo_blk)
        bo2_sb = consts.tile([128, G * D], F32, tag="bo2_sb")
        nc.sync.dma_start(bo2_sb[:], bo2_rep)

        # v33 ring: [t, (c,33)] with ones in col 32 of each 33-block
        v33_tiles = []
        for i in range(4):
            t = v33_pool.tile([128, G * 33], BF16, tag=f"v33_{i}")
            ones_ap = t[:].rearrange("p (c g) -> p c g", g=33)[:, :, 32:33]
            nc.vector.memset(ones_ap, 1.0)
            v33_tiles.append(t)

        g_global = 0
        for sc in range(N_SC):
            r0 = sc * 128
            # input slabs: [(c4 f)=128, (j s)=1024], contiguous 2KB rows
            hg_sc = io_pool.tile([128, SLAB_COLS], BF16, tag="hg_sc")
            hp_sc = io_pool.tile([128, SLAB_COLS], BF16, tag="hp_sc")
            for h in range(2):
                half = slice(h * 512, (h + 1) * 512)
                nc.sync.dma_start(hg_sc[:, half], hg_p[r0 : r0 + 128, half])
                nc.sync.dma_start(hp_sc[:, half], hp_p[r0 : r0 + 128, half])
            out_sc = io_pool.tile([128, SC_CLUSTERS * D], F32, tag="out_sc")

            # Z phase: Z'^T slab [(c,f), (j,s)] via 2 wide matmuls + ACT evac
            z_sb = zsb_pool.tile([128, SLAB_COLS], BF16, tag="z_sb")
            for h in range(2):
                half = slice(h * 512, (h + 1) * 512)
                z_ps = ps_z.tile([128, 512], F32, tag="z_ps")
                nc.tensor.matmul(z_ps[:], a_sb[:], hg_sc[:, half])
                nc.scalar.activation(
                    z_sb[:, half], z_ps[:],
                    mybir.ActivationFunctionType.Identity, bias=cvec_sb[:],
                )

            f2 = None
            for j in range(GROUPS_PER_SC):
                jcols = slice(j * S, (j + 1) * S)
                xg = hg_sc[:, jcols]
                xp = hp_sc[:, jcols]
                zj = z_sb[:, jcols]

                # S^T[t,s] = Xg Z'^T: 4 row-banded matmuls, one bank per band
                wk = ps_work.tile([128, 2048], F32, tag="wk")
                for c in range(G):
                    p0 = c * 32
                    nc.tensor.matmul(
                        wk[:, c * 512 : c * 512 + 128],
                        xg[p0 : p0 + 32, :],
                        zj[p0 : p0 + 32, :],
                        tile_position=(p0, 0),
                    )
                wk_view = wk[:].rearrange("p (c q) -> p c q", q=512)
                p_sb = p_pool.tile([128, 512], BF16, tag="p_sb")
                nc.scalar.activation(
                    p_sb[:].rearrange("p (c q) -> p c q", q=128),
                    wk_view[:, :, 0:128],
                    mybir.ActivationFunctionType.Exp,
                )

                # V'[t,(c,g)] = Xp blockdiag(Wvo^T): one matmul
                v_ps = ps_v.tile([128, 128], F32, tag="v_ps")
                nc.tensor.matmul(v_ps[:], xp, wvo_sb[:])
                # V'' = V' + bo2 -> bf16, strided into the v33 ring
                v33 = v33_tiles[g_global % 4]
                nc.vector.tensor_tensor(
                    v33[:].rearrange("p (c g) -> p c g", g=33)[:, :, 0:32],
                    v_ps[:].rearrange("p (c g) -> p c g", g=D),
                    bo2_sb[:].rearrange("p (c g) -> p c g", g=D),
                    mybir.AluOpType.add,
                )

                # F_un[s,(c,33)] = P^T.T @ [V''|1]; col 32 of block = r[s]
                # two groups share one PSUM tile so recip/mult batch 8 blocks
                if j % 2 == 0:
                    f2 = ps_f.tile([128, 2 * G * 33], F32, tag="f2")
                fbase = (j % 2) * G * 33
                for c in range(G):
                    nc.tensor.matmul(
                        f2[:, fbase + c * 33 : fbase + (c + 1) * 33],
                        p_sb[:, c * 128 : (c + 1) * 128],
                        v33[:, c * 33 : (c + 1) * 33],
                        tile_position=(0, 0),
                    )
                g_global += 1

                if j % 2 == 1:
                    f_view = f2[:].rearrange("p (c g) -> p c g", g=33)
                    recip = small_pool.tile([128, 2 * G], F32, tag="recip")
                    nc.vector.reciprocal(recip[:, :, None], f_view[:, :, 32:33])
                    nc.vector.tensor_tensor(
                        out_sc[:, (j - 1) * 128 : (j + 1) * 128].rearrange(
                            "p (c d) -> p c d", d=D
                        ),
                        f_view[:, :, 0:32],
                        recip[:, :, None].to_broadcast([128, 2 * G, D]),
                        mybir.AluOpType.mult,
                    )

            # store the superchunk: contiguous 4KB partition rows
            nc.sync.dma_start(
                out[:, sc * SC_CLUSTERS * D : (sc + 1) * SC_CLUSTERS * D],
                out_sc[:],
            )

    nc.compile()
    return nc


_PROGRAM = None


def _get_program():
    global _PROGRAM
    if _PROGRAM is None:
        _PROGRAM = _build_program()
    return _PROGRAM


def _host_fold(Wq, bq, Wk, bk, Wv, bv, Wo, bo):
    Wq64, Wk64 = np.asarray(Wq, np.float64), np.asarray(Wk, np.float64)
    Wv64, Wo64 = np.asarray(Wv, np.float64), np.asarray(Wo, np.float64)
    bq64, bv64, bo64 = (np.asarray(x, np.float64) for x in (bq, bv, bo))
    scale = 1.0 / np.sqrt(np.float64(D))
    A = (Wq64.T @ Wk64) * scale                      # [e, f]
    c = (bq64 @ Wk64) * scale                        # [f]
    WvoT = (Wo64 @ Wv64).T                           # [e, g]
    bo2 = bo64 + Wo64 @ bv64                         # [g]
    a_blk = np.zeros((128, 128), NP_BF16)
    wvo_blk = np.zeros((128, 128), NP_BF16)
    for cc in range(G):
        a_blk[cc * D : (cc + 1) * D, cc * D : (cc + 1) * D] = A.astype(NP_BF16)
        wvo_blk[cc * D : (cc + 1) * D, cc * D : (cc + 1) * D] = WvoT.astype(
            NP_BF16
        )
    cvec = np.tile(c, G)[:, None].astype(np.float32)         # [128, 1]
    bo2_rep = np.tile(bo2, (128, G)).reshape(128, G * D).astype(np.float32)
    return a_blk, cvec, wvo_blk, bo2_rep


def _pack_slabs(x):
    """[B, N, D] -> per-core [N_SC*128, 1024] bf16 slab layout."""
    # [b, cblk, j, c4, s, f] -> [b, cblk, c4, f, j, s]
    y = np.asarray(x, np.float32).reshape(B, 4, GROUPS_PER_SC, G, S, D)
    y = y.transpose(0, 1, 3, 5, 2, 4).astype(NP_BF16)
    return y.reshape(B, 4 * 128, SLAB_COLS)  # [b, cblk*(c4 f), (j s)]


def make_in_maps(h_pos, h_geo, Wq, bq, Wk, bk, Wv, bv, Wo, bo):
    a_blk, cvec, wvo_blk, bo2_rep = _host_fold(Wq, bq, Wk, bk, Wv, bv, Wo, bo)
    hg_all = _pack_slabs(h_geo)
    hp_all = _pack_slabs(h_pos)
    in_maps = []
    for core in range(N_CORES):
        b0 = core * B_LOC
        in_maps.append(
            {
                "hg_p": np.ascontiguousarray(
                    hg_all[b0 : b0 + B_LOC].reshape(N_SC * 128, SLAB_COLS)
                ),
                "hp_p": np.ascontiguousarray(
                    hp_all[b0 : b0 + B_LOC].reshape(N_SC * 128, SLAB_COLS)
                ),
                "a_blk": a_blk,
                "cvec": cvec,
                "wvo_blk": wvo_blk,
                "bo2_rep": bo2_rep,
            }
        )
    return in_maps


def _unpack_out(out_dram):
    """[128, 8192] fp32 -> [B_LOC, N, D]."""
    y = out_dram.reshape(S, N_SC, GROUPS_PER_SC, G, D)
    y = y.transpose(1, 2, 3, 0, 4)  # [sc, j, c, s, d]
    return y.reshape(B_LOC, N, D)


def kernel(h_pos, h_geo, n_clusters, Wq, bq, Wk, bk, Wv, bv, Wo, bo, **kwargs):
    assert int(n_clusters) == C_TOTAL
    nc = _get_program()
    in_maps = make_in_maps(h_pos, h_geo, Wq, bq, Wk, bk, Wv, bv, Wo, bo)
    res = run_bass_kernel_spmd(nc, in_maps, core_ids=list(range(N_CORES)))
    shards = [_unpack_out(r["out"]) for r in res.results]
    return np.concatenate(shards, axis=0).astype(np.float32)


# revision 5
# speedup vs baseline: 2.9315x; 1.3273x over previous
"""DLSA block (clustered sparse attention) Trainium2 kernel.

Full-input contract: kernel(**inputs) takes the complete unsharded tensors,
shards batch-dim across 8 NeuronCores, runs a Bass/Tile kernel per core, and
gathers the full output on host.

v3: bf16 datapath + software-pipelined pair loop.
  - All matmul operands bf16 (fp32 PSUM accumulate), rel err ~6e-3.
  - Scores for TWO groups share one 4-bank PSUM generation (8 row-banded
    matmuls; bank c holds clusters c of both groups at col offsets 0/128),
    so there is ONE exp ACTIVATE per 8 clusters and the scalar engine runs
    only exp (~33us). The exp->next-scores WAR dependency is covered by
    issuing V matmuls, the previous pair's F matmuls, and the Z matmuls of
    the next superchunk inside the window.
  - Normalization moved to host: F is stored UNNORMALIZED together with the
    softmax denominator column (33 cols per cluster); the unpack step does
    out = F/r. On-device vector work is only z-evac, V bias-add, F evac.
  - Host packs inputs into slab layout ([sc, (c4 f)=128, (j s)=1024] bf16,
    2KB DMA rows); output leaves as [s=128, (sc, pair, g, c, 33)] fp32.

Algebraic folds done on host (weight-space only, float64 for accuracy):
  A    = Wq^T @ Wk / sqrt(D)      -> scores S = Xg A Xg^T + (bq Wk/sqrt(D)) Xg^T
  bk drops entirely (softmax-invariant row constant).
  Wvo  = Wo @ Wv                  -> V' = Xp Wvo^T  (V and O projections fused)
  bo2  = bo + Wo @ bv             (bv commutes through attention; added to V')
"""

import sys

for _p in ("/opt/trn_rl_repo",):
    if _p not in sys.path:
        sys.path.insert(0, _p)

from contextlib import ExitStack

import numpy as np
import ml_dtypes

import concourse.bass as bass
import concourse.tile as tile
from concourse import bacc, mybir
from concourse.bass_utils import run_bass_kernel_spmd

F32 = mybir.dt.float32
BF16 = mybir.dt.bfloat16
NP_BF16 = ml_dtypes.bfloat16

B, N, D = 16, 16384, 32
C_TOTAL, S = 128, 128          # clusters per batch, points per cluster
N_CORES = 8
B_LOC = B // N_CORES           # batches per core
ROWS = B_LOC * N               # data rows per core
SC_CLUSTERS = 32               # clusters per superchunk
N_SC = ROWS // (SC_CLUSTERS * S)  # 8 superchunks per core
G = 4                          # clusters per group
GROUPS_PER_SC = SC_CLUSTERS // G  # 8
SLAB_COLS = GROUPS_PER_SC * S  # 1024 cols per input slab
PAIRS_PER_SC = GROUPS_PER_SC // 2  # 4
N_PAIRS = N_SC * PAIRS_PER_SC  # 32 pairs per core
FCOLS = 2 * G * 33             # 264 f-cols per pair (8 clusters x [32 vals | r])


def _build_program():
    nc = bacc.Bacc("TRN2", target_bir_lowering=False, debug=False)

    hg_p = nc.dram_tensor("hg_p", [N_SC * 128, SLAB_COLS], BF16,
                          kind="ExternalInput").ap()
    hp_p = nc.dram_tensor("hp_p", [N_SC * 128, SLAB_COLS], BF16,
                          kind="ExternalInput").ap()
    a_blk = nc.dram_tensor("a_blk", [128, 128], BF16, kind="ExternalInput").ap()
    cvec = nc.dram_tensor("cvec", [128, 1], F32, kind="ExternalInput").ap()
    wvo_blk = nc.dram_tensor("wvo_blk", [128, 128], BF16,
                             kind="ExternalInput").ap()
    bo2_rep = nc.dram_tensor("bo2_rep", [128, 2 * G * D], F32,
                             kind="ExternalInput").ap()
    out = nc.dram_tensor("out", [128, N_PAIRS * FCOLS], F32,
                         kind="ExternalOutput").ap()

    with tile.TileContext(nc) as tc, ExitStack() as ctx:
        consts = ctx.enter_context(tc.tile_pool(name="consts", bufs=1))
        io_pool = ctx.enter_context(tc.tile_pool(name="io", bufs=2))
        zsb_pool = ctx.enter_context(tc.tile_pool(name="zsb", bufs=2))
        p_pool = ctx.enter_context(tc.tile_pool(name="p", bufs=2))
        out_pool = ctx.enter_context(tc.tile_pool(name="osc", bufs=2))
        v33_pool = ctx.enter_context(tc.tile_pool(name="v33", bufs=2))

        # PSUM: 8 banks of 2KB. wk 4 (one per score row band, two groups per
        # generation at col offsets 0/128) + z 1 + v 1 + f 2.
        ps_z = ctx.enter_context(tc.tile_pool(name="ps_z", bufs=1, space="PSUM"))
        ps_wk = ctx.enter_context(tc.tile_pool(name="ps_wk", bufs=1, space="PSUM"))
        ps_v = ctx.enter_context(tc.tile_pool(name="ps_v", bufs=1, space="PSUM"))
        ps_f = ctx.enter_context(tc.tile_pool(name="ps_f", bufs=2, space="PSUM"))

        # constants
        a_sb = consts.tile([128, 128], BF16, tag="a_sb")
        nc.sync.dma_start(a_sb[:], a_blk)
        cvec_sb = consts.tile([128, 1], F32, tag="cvec_sb")
        nc.sync.dma_start(cvec_sb[:], cvec)
        wvo_sb = consts.tile([128, 128], BF16, tag="wvo_sb")
        nc.sync.dma_start(wvo_sb[:], wvo_blk)
        bo2_sb = consts.tile([128, 2 * G * D], F32, tag="bo2_sb")
        nc.sync.dma_start(bo2_sb[:], bo2_rep)

        # warm the exp table while the first slabs load
        warm = consts.tile([128, 1], F32, tag="warm")
        nc.scalar.activation(warm[:], cvec_sb[:],
                             mybir.ActivationFunctionType.Exp)

        # v33 ring: [t, (g c 33)] with ones in col 32 of each 33-block
        v33_tiles = []
        for i in range(2):
            t = v33_pool.tile([128, FCOLS], BF16, tag=f"v33_{i}")
            ones_ap = t[:].rearrange("p (c g) -> p c g", g=33)[:, :, 32:33]
            nc.gpsimd.memset(ones_ap, 1.0)
            v33_tiles.append(t)

        slabs = {}

        def load_slabs(sc):
            hg_sc = io_pool.tile([128, SLAB_COLS], BF16, tag="hg_sc")
            hp_sc = io_pool.tile([128, SLAB_COLS], BF16, tag="hp_sc")
            r0 = sc * 128
            nc.sync.dma_start(hg_sc[:], hg_p[r0 : r0 + 128, :])
            nc.sync.dma_start(hp_sc[:], hp_p[r0 : r0 + 128, :])
            slabs[sc] = (hg_sc, hp_sc)

        zsb = {}

        def z_phase(sc, h):
            """Z'^T half-slab via one wide matmul + DVE evac (bias cvec)."""
            hg_sc, _ = slabs[sc]
            if h == 0:
                zsb[sc] = zsb_pool.tile([128, SLAB_COLS], BF16, tag="z_sb", name="z_sb")
            half = slice(h * 512, (h + 1) * 512)
            z_ps = ps_z.tile([128, 512], F32, tag="z_ps")
            nc.tensor.matmul(z_ps[:], a_sb[:], hg_sc[:, half])
            nc.vector.tensor_scalar_add(zsb[sc][:, half], z_ps[:], cvec_sb[:])

        # state carried across pairs for the lag-1 F stage
        prev = None  # (p_sb2, v33, f2 slot filled later)
        f_tiles = {}
        out_tiles = {}

        def emit_F(P):
            """F matmuls for pair P (reads p_sb2/v33 of P), into f2[P]."""
            p_sb2, v33 = prev_state[P]
            f2 = ps_f.tile([128, FCOLS], F32, tag="f2", name="f2")
            f_tiles[P] = f2
            for g in range(2):
                for c in range(G):
                    col = (g * G + c) * 33
                    nc.tensor.matmul(
                        f2[:, col : col + 33],
                        p_sb2[:, c * 256 + g * 128 : c * 256 + (g + 1) * 128],
                        v33[:, col : col + 33],
                        tile_position=(0, 0),
                    )

        def emit_F_evac(P):
            """Copy f2[P] (unnormalized F + r cols) into the sc out tile."""
            sc = P // PAIRS_PER_SC
            p = P % PAIRS_PER_SC
            if p == 0:
                out_tiles[sc] = out_pool.tile([128, PAIRS_PER_SC * FCOLS],
                                              F32, tag="out_sc", name="out_sc")
            nc.vector.tensor_copy(
                out_tiles[sc][:, p * FCOLS : (p + 1) * FCOLS],
                f_tiles[P][:],
            )
            del f_tiles[P]

        def store_out(sc):
            nc.sync.dma_start(
                out[:, sc * PAIRS_PER_SC * FCOLS : (sc + 1) * PAIRS_PER_SC * FCOLS],
                out_tiles[sc][:],
            )

        prev_state = {}

        # prologue
        load_slabs(0)
        z_phase(0, 0)
        z_phase(0, 1)

        for P in range(N_PAIRS):
            sc, p = P // PAIRS_PER_SC, P % PAIRS_PER_SC
            hg_sc, hp_sc = slabs[sc]
            z_sb = zsb[sc]
            if p == 0 and sc + 1 < N_SC:
                load_slabs(sc + 1)

            # scores for both groups of the pair: 8 row-banded matmuls.
            # bank c (cols c*512..) holds cluster c of group g at +g*128.
            wk = ps_wk.tile([128, 2048], F32, tag="wk")
            for g in range(2):
                j = p * 2 + g
                jcols = slice(j * S, (j + 1) * S)
                for c in range(G):
                    p0 = c * 32
                    nc.tensor.matmul(
                        wk[:, c * 512 + g * 128 : c * 512 + (g + 1) * 128],
                        hg_sc[p0 : p0 + 32, jcols],
                        z_sb[p0 : p0 + 32, jcols],
                        tile_position=(p0, 0),
                    )

            # one exp per pair: [128, (c, g2, s)=1024] -> bf16
            wk_view = wk[:].rearrange("p (c q) -> p c q", q=512)
            p_sb2 = p_pool.tile([128, 1024], BF16, tag="p_sb2")
            nc.scalar.activation(
                p_sb2[:].rearrange("p (c q) -> p c q", q=256),
                wk_view[:, :, 0:256],
                mybir.ActivationFunctionType.Exp,
            )

            # V' for both groups -> v_ps [128, (g, c, 32)]
            v_ps = ps_v.tile([128, 256], F32, tag="v_ps")
            for g in range(2):
                j = p * 2 + g
                nc.tensor.matmul(
                    v_ps[:, g * 128 : (g + 1) * 128],
                    hp_sc[:, (j * S) : (j + 1) * S],
                    wvo_sb[:],
                )

            # lag-1 F matmuls fill the exp->scores window
            if P > 0:
                emit_F(P - 1)

            # Z matmuls of the next superchunk also land in the window
            if p == 2 and sc + 1 < N_SC:
                z_phase(sc + 1, 0)
            elif p == 3 and sc + 1 < N_SC:
                z_phase(sc + 1, 1)

            # V'' = V' + bo2 -> bf16 into the v33 ring (ones col preserved)
            v33 = v33_tiles[P % 2]
            nc.vector.tensor_tensor(
                v33[:].rearrange("p (c g) -> p c g", g=33)[:, :, 0:32],
                v_ps[:].rearrange("p (c g) -> p c g", g=D),
                bo2_sb[:].rearrange("p (c g) -> p c g", g=D),
                mybir.AluOpType.add,
            )
            prev_state[P] = (p_sb2, v33)

            if P > 0:
                emit_F_evac(P - 1)
                if P % PAIRS_PER_SC == 0:
                    store_out(sc - 1)
                del prev_state[P - 1]

        # epilogue
        emit_F(N_PAIRS - 1)
        emit_F_evac(N_PAIRS - 1)
        store_out(N_SC - 1)

    nc.compile()
    return nc


_PROGRAM = None


def _get_program():
    global _PROGRAM
    if _PROGRAM is None:
        _PROGRAM = _build_program()
    return _PROGRAM


def _host_fold(Wq, bq, Wk, bk, Wv, bv, Wo, bo):
    Wq64, Wk64 = np.asarray(Wq, np.float64), np.asarray(Wk, np.float64)
    Wv64, Wo64 = np.asarray(Wv, np.float64), np.asarray(Wo, np.float64)
    bq64, bv64, bo64 = (np.asarray(x, np.float64) for x in (bq, bv, bo))
    scale = 1.0 / np.sqrt(np.float64(D))
    A = (Wq64.T @ Wk64) * scale                      # [e, f]
    c = (bq64 @ Wk64) * scale                        # [f]
    WvoT = (Wo64 @ Wv64).T                           # [e, g]
    bo2 = bo64 + Wo64 @ bv64                         # [g]
    a_blk = np.zeros((128, 128), NP_BF16)
    wvo_blk = np.zeros((128, 128), NP_BF16)
    for cc in range(G):
        a_blk[cc * D : (cc + 1) * D, cc * D : (cc + 1) * D] = A.astype(NP_BF16)
        wvo_blk[cc * D : (cc + 1) * D, cc * D : (cc + 1) * D] = WvoT.astype(
            NP_BF16
        )
    cvec = np.tile(c, G)[:, None].astype(np.float32)         # [128, 1]
    bo2_rep = np.tile(bo2, (128, 2 * G)).reshape(128, 2 * G * D).astype(
        np.float32
    )
    return a_blk, cvec, wvo_blk, bo2_rep


def _pack_slabs(x):
    """[B, N, D] -> per-batch [4*128, 1024] bf16 slab layout."""
    # [b, cblk, j, c4, s, f] -> [b, cblk, c4, f, j, s]
    y = np.asarray(x, np.float32).reshape(B, 4, GROUPS_PER_SC, G, S, D)
    y = y.transpose(0, 1, 3, 5, 2, 4).astype(NP_BF16)
    return y.reshape(B, 4 * 128, SLAB_COLS)


def make_in_maps(h_pos, h_geo, Wq, bq, Wk, bk, Wv, bv, Wo, bo):
    a_blk, cvec, wvo_blk, bo2_rep = _host_fold(Wq, bq, Wk, bk, Wv, bv, Wo, bo)
    hg_all = _pack_slabs(h_geo)
    hp_all = _pack_slabs(h_pos)
    in_maps = []
    for core in range(N_CORES):
        b0 = core * B_LOC
        in_maps.append(
            {
                "hg_p": np.ascontiguousarray(
                    hg_all[b0 : b0 + B_LOC].reshape(N_SC * 128, SLAB_COLS)
                ),
                "hp_p": np.ascontiguousarray(
                    hp_all[b0 : b0 + B_LOC].reshape(N_SC * 128, SLAB_COLS)
                ),
                "a_blk": a_blk,
                "cvec": cvec,
                "wvo_blk": wvo_blk,
                "bo2_rep": bo2_rep,
            }
        )
    return in_maps


def _unpack_out(out_dram):
    """[128, N_PAIRS*264] fp32 unnormalized -> [B_LOC, N, D] (host divide)."""
    y = out_dram.reshape(S, N_PAIRS, 2, G, 33)  # [s, pair, g, c, 33]
    f = y[..., :32]
    r = y[..., 32:33]
    o = f / r                                   # softmax normalization
    # pair,g -> group j (2 per pair); cluster = (sc, j, c)
    o = o.transpose(1, 2, 3, 0, 4)              # [pair, g, c, s, d]
    return o.reshape(B_LOC, N, D)


def kernel(h_pos, h_geo, n_clusters, Wq, bq, Wk, bk, Wv, bv, Wo, bo, **kwargs):
    assert int(n_clusters) == C_TOTAL
    nc = _get_program()
    in_maps = make_in_maps(h_pos, h_geo, Wq, bq, Wk, bk, Wv, bv, Wo, bo)
    res = run_bass_kernel_spmd(nc, in_maps, core_ids=list(range(N_CORES)))
    shards = [_unpack_out(r["out"]) for r in res.results]
    return np.concatenate(shards, axis=0).astype(np.float32)


# revision 6
# speedup vs baseline: 3.3624x; 1.1470x over previous
"""DLSA block (clustered sparse attention) Trainium2 kernel.

Full-input contract: kernel(**inputs) takes the complete unsharded tensors,
shards batch-dim across 8 NeuronCores, runs a Bass/Tile kernel per core, and
gathers the full output on host.

v3: bf16 datapath + software-pipelined pair loop.
  - All matmul operands bf16 (fp32 PSUM accumulate), rel err ~6e-3.
  - Scores for TWO groups share one 4-bank PSUM generation (8 row-banded
    matmuls; bank c holds clusters c of both groups at col offsets 0/128),
    so there is ONE exp ACTIVATE per 8 clusters and the scalar engine runs
    only exp (~33us). The exp->next-scores WAR dependency is covered by
    issuing V matmuls, the previous pair's F matmuls, and the Z matmuls of
    the next superchunk inside the window.
  - Normalization moved to host: F is stored UNNORMALIZED together with the
    softmax denominator column (33 cols per cluster); the unpack step does
    out = F/r. On-device vector work is only z-evac, V bias-add, F evac.
  - Host packs inputs into slab layout ([sc, (c4 f)=128, (j s)=1024] bf16,
    2KB DMA rows); output leaves as [s=128, (sc, pair, g, c, 33)] fp32.

Algebraic folds done on host (weight-space only, float64 for accuracy):
  A    = Wq^T @ Wk / sqrt(D)      -> scores S = Xg A Xg^T + (bq Wk/sqrt(D)) Xg^T
  bk drops entirely (softmax-invariant row constant).
  Wvo  = Wo @ Wv                  -> V' = Xp Wvo^T  (V and O projections fused)
  bo2  = bo + Wo @ bv             (bv commutes through attention; added to V')
"""

import sys

for _p in ("/opt/trn_rl_repo",):
    if _p not in sys.path:
        sys.path.insert(0, _p)

from contextlib import ExitStack

import numpy as np
import ml_dtypes

import concourse.bass as bass
import concourse.tile as tile
from concourse import bacc, mybir
from concourse.bass_utils import run_bass_kernel_spmd

F32 = mybir.dt.float32
BF16 = mybir.dt.bfloat16
NP_BF16 = ml_dtypes.bfloat16

B, N, D = 16, 16384, 32
C_TOTAL, S = 128, 128          # clusters per batch, points per cluster
N_CORES = 8
B_LOC = B // N_CORES           # batches per core
ROWS = B_LOC * N               # data rows per core
SC_CLUSTERS = 32               # clusters per superchunk
N_SC = ROWS // (SC_CLUSTERS * S)  # 8 superchunks per core
G = 4                          # clusters per group
GROUPS_PER_SC = SC_CLUSTERS // G  # 8
SLAB_COLS = GROUPS_PER_SC * S  # 1024 cols per input slab
PAIRS_PER_SC = GROUPS_PER_SC // 2  # 4
N_PAIRS = N_SC * PAIRS_PER_SC  # 32 pairs per core
FCOLS = 2 * G * 33             # 264 f-cols per pair (8 clusters x [32 vals | r])


def _build_program():
    nc = bacc.Bacc("TRN2", target_bir_lowering=False, debug=False)

    hg_p = nc.dram_tensor("hg_p", [N_SC * 128, SLAB_COLS], BF16,
                          kind="ExternalInput").ap()
    hp_p = nc.dram_tensor("hp_p", [N_SC * 128, SLAB_COLS], BF16,
                          kind="ExternalInput").ap()
    a_blk = nc.dram_tensor("a_blk", [128, 128], BF16, kind="ExternalInput").ap()
    cvec = nc.dram_tensor("cvec", [128, 1], F32, kind="ExternalInput").ap()
    wvo_blk = nc.dram_tensor("wvo_blk", [128, 128], BF16,
                             kind="ExternalInput").ap()
    bo2_rep = nc.dram_tensor("bo2_rep", [128, 2 * G * D], F32,
                             kind="ExternalInput").ap()
    out = nc.dram_tensor("out", [128, N_PAIRS * FCOLS], F32,
                         kind="ExternalOutput").ap()

    with tile.TileContext(nc) as tc, ExitStack() as ctx:
        consts = ctx.enter_context(tc.tile_pool(name="consts", bufs=1))
        io_pool = ctx.enter_context(tc.tile_pool(name="io", bufs=2))
        zsb_pool = ctx.enter_context(tc.tile_pool(name="zsb", bufs=2))
        pA_pool = ctx.enter_context(tc.tile_pool(name="pA", bufs=2))
        pB_pool = ctx.enter_context(tc.tile_pool(name="pB", bufs=2))
        out_pool = ctx.enter_context(tc.tile_pool(name="osc", bufs=2))
        v33_pool = ctx.enter_context(tc.tile_pool(name="v33", bufs=2))

        # PSUM: 8 banks of 2KB. Scores split into two 2-bank generations
        # (bands c0,c1 -> pool A; c2,c3 -> pool B) so the next pair's A-half
        # can run while expB of this pair still drains: A2 + B2 + z1 + v1 +
        # f2 = 8 banks.
        ps_z = ctx.enter_context(tc.tile_pool(name="ps_z", bufs=1, space="PSUM"))
        ps_wkA = ctx.enter_context(tc.tile_pool(name="ps_wkA", bufs=1, space="PSUM"))
        ps_wkB = ctx.enter_context(tc.tile_pool(name="ps_wkB", bufs=1, space="PSUM"))
        ps_v = ctx.enter_context(tc.tile_pool(name="ps_v", bufs=1, space="PSUM"))
        ps_f = ctx.enter_context(tc.tile_pool(name="ps_f", bufs=2, space="PSUM"))

        # constants
        a_sb = consts.tile([128, 128], BF16, tag="a_sb")
        nc.sync.dma_start(a_sb[:], a_blk)
        cvec_sb = consts.tile([128, 1], F32, tag="cvec_sb")
        nc.sync.dma_start(cvec_sb[:], cvec)
        wvo_sb = consts.tile([128, 128], BF16, tag="wvo_sb")
        nc.sync.dma_start(wvo_sb[:], wvo_blk)
        bo2_sb = consts.tile([128, 2 * G * D], F32, tag="bo2_sb")
        nc.sync.dma_start(bo2_sb[:], bo2_rep)

        # warm the exp table while the first slabs load
        warm = consts.tile([128, 1], F32, tag="warm")
        nc.scalar.activation(warm[:], cvec_sb[:],
                             mybir.ActivationFunctionType.Exp)

        # v33 ring: [t, (g c 33)] with ones in col 32 of each 33-block
        v33_tiles = []
        for i in range(2):
            t = v33_pool.tile([128, FCOLS], BF16, tag=f"v33_{i}")
            ones_ap = t[:].rearrange("p (c g) -> p c g", g=33)[:, :, 32:33]
            nc.gpsimd.memset(ones_ap, 1.0)
            v33_tiles.append(t)

        slabs = {}

        def load_slabs(sc):
            hg_sc = io_pool.tile([128, SLAB_COLS], BF16, tag="hg_sc")
            hp_sc = io_pool.tile([128, SLAB_COLS], BF16, tag="hp_sc")
            r0 = sc * 128
            nc.sync.dma_start(hg_sc[:], hg_p[r0 : r0 + 128, :])
            nc.sync.dma_start(hp_sc[:], hp_p[r0 : r0 + 128, :])
            slabs[sc] = (hg_sc, hp_sc)

        zsb = {}

        def z_phase(sc, h):
            """Z'^T half-slab via one wide matmul + DVE evac (bias cvec)."""
            hg_sc, _ = slabs[sc]
            if h == 0:
                zsb[sc] = zsb_pool.tile([128, SLAB_COLS], BF16, tag="z_sb", name="z_sb")
            half = slice(h * 512, (h + 1) * 512)
            z_ps = ps_z.tile([128, 512], F32, tag="z_ps")
            nc.tensor.matmul(z_ps[:], a_sb[:], hg_sc[:, half])
            nc.vector.tensor_scalar_add(zsb[sc][:, half], z_ps[:], cvec_sb[:])

        # state carried across pairs for the lag-1 F stage
        prev = None  # (p_sb2, v33, f2 slot filled later)
        f_tiles = {}
        out_tiles = {}

        def emit_F(P):
            """F matmuls for pair P (reads p_sb2/v33 of P), into f2[P]."""
            p_sbA, p_sbB, v33 = prev_state[P]
            f2 = ps_f.tile([128, FCOLS], F32, tag="f2", name="f2")
            f_tiles[P] = f2
            for g in range(2):
                for c in range(G):
                    col = (g * G + c) * 33
                    p_sb = p_sbA if c < 2 else p_sbB
                    ci = c % 2
                    nc.tensor.matmul(
                        f2[:, col : col + 33],
                        p_sb[:, ci * 256 + g * 128 : ci * 256 + (g + 1) * 128],
                        v33[:, col : col + 33],
                        tile_position=(0, 0),
                    )

        def emit_F_evac(P):
            """Copy f2[P] (unnormalized F + r cols) into the sc out tile."""
            sc = P // PAIRS_PER_SC
            p = P % PAIRS_PER_SC
            if p == 0:
                out_tiles[sc] = out_pool.tile([128, PAIRS_PER_SC * FCOLS],
                                              F32, tag="out_sc", name="out_sc")
            nc.vector.tensor_copy(
                out_tiles[sc][:, p * FCOLS : (p + 1) * FCOLS],
                f_tiles[P][:],
            )
            del f_tiles[P]

        def store_out(sc):
            nc.sync.dma_start(
                out[:, sc * PAIRS_PER_SC * FCOLS : (sc + 1) * PAIRS_PER_SC * FCOLS],
                out_tiles[sc][:],
            )

        prev_state = {}

        # prologue
        load_slabs(0)
        z_phase(0, 0)
        z_phase(0, 1)

        for P in range(N_PAIRS):
            sc, p = P // PAIRS_PER_SC, P % PAIRS_PER_SC
            hg_sc, hp_sc = slabs[sc]
            z_sb = zsb[sc]
            if p == 0 and sc + 1 < N_SC:
                load_slabs(sc + 1)

            # scores: 8 row-banded matmuls in two 2-bank halves.
            # Half A = bands c0,c1; half B = bands c2,c3; within a half,
            # band i (cols i*512..) holds group g at +g*128.
            p_half = []
            for half_i, (ps_pool, p_pool, c_lo) in enumerate(
                ((ps_wkA, pA_pool, 0), (ps_wkB, pB_pool, 2))
            ):
                wk = ps_pool.tile([128, 1024], F32, tag="wk", name="wk")
                for g in range(2):
                    j = p * 2 + g
                    jcols = slice(j * S, (j + 1) * S)
                    for ci in range(2):
                        c = c_lo + ci
                        p0 = c * 32
                        nc.tensor.matmul(
                            wk[:, ci * 512 + g * 128 : ci * 512 + (g + 1) * 128],
                            hg_sc[p0 : p0 + 32, jcols],
                            z_sb[p0 : p0 + 32, jcols],
                            tile_position=(p0, 0),
                        )
                wk_view = wk[:].rearrange("p (c q) -> p c q", q=512)
                p_sb = p_pool.tile([128, 512], BF16, tag="p_sb", name="p_sb")
                nc.scalar.activation(
                    p_sb[:].rearrange("p (c q) -> p c q", q=256),
                    wk_view[:, :, 0:256],
                    mybir.ActivationFunctionType.Exp,
                )
                p_half.append(p_sb)

            # V' for both groups -> v_ps [128, (g, c, 32)]
            v_ps = ps_v.tile([128, 256], F32, tag="v_ps")
            for g in range(2):
                j = p * 2 + g
                nc.tensor.matmul(
                    v_ps[:, g * 128 : (g + 1) * 128],
                    hp_sc[:, (j * S) : (j + 1) * S],
                    wvo_sb[:],
                )

            # lag-1 F matmuls fill the exp->scores window
            if P > 0:
                emit_F(P - 1)

            # Z matmuls of the next superchunk also land in the window
            if p == 2 and sc + 1 < N_SC:
                z_phase(sc + 1, 0)
            elif p == 3 and sc + 1 < N_SC:
                z_phase(sc + 1, 1)

            # V'' = V' + bo2 -> bf16 into the v33 ring (ones col preserved)
            v33 = v33_tiles[P % 2]
            nc.vector.tensor_tensor(
                v33[:].rearrange("p (c g) -> p c g", g=33)[:, :, 0:32],
                v_ps[:].rearrange("p (c g) -> p c g", g=D),
                bo2_sb[:].rearrange("p (c g) -> p c g", g=D),
                mybir.AluOpType.add,
            )
            prev_state[P] = (p_half[0], p_half[1], v33)

            if P > 0:
                emit_F_evac(P - 1)
                if P % PAIRS_PER_SC == 0:
                    store_out(sc - 1)
                del prev_state[P - 1]

        # epilogue
        emit_F(N_PAIRS - 1)
        emit_F_evac(N_PAIRS - 1)
        store_out(N_SC - 1)

    nc.compile()
    return nc


_PROGRAM = None


def _get_program():
    global _PROGRAM
    if _PROGRAM is None:
        _PROGRAM = _build_program()
    return _PROGRAM


def _host_fold(Wq, bq, Wk, bk, Wv, bv, Wo, bo):
    Wq64, Wk64 = np.asarray(Wq, np.float64), np.asarray(Wk, np.float64)
    Wv64, Wo64 = np.asarray(Wv, np.float64), np.asarray(Wo, np.float64)
    bq64, bv64, bo64 = (np.asarray(x, np.float64) for x in (bq, bv, bo))
    scale = 1.0 / np.sqrt(np.float64(D))
    A = (Wq64.T @ Wk64) * scale                      # [e, f]
    c = (bq64 @ Wk64) * scale                        # [f]
    WvoT = (Wo64 @ Wv64).T                           # [e, g]
    bo2 = bo64 + Wo64 @ bv64                         # [g]
    a_blk = np.zeros((128, 128), NP_BF16)
    wvo_blk = np.zeros((128, 128), NP_BF16)
    for cc in range(G):
        a_blk[cc * D : (cc + 1) * D, cc * D : (cc + 1) * D] = A.astype(NP_BF16)
        wvo_blk[cc * D : (cc + 1) * D, cc * D : (cc + 1) * D] = WvoT.astype(
            NP_BF16
        )
    cvec = np.tile(c, G)[:, None].astype(np.float32)         # [128, 1]
    bo2_rep = np.tile(bo2, (128, 2 * G)).reshape(128, 2 * G * D).astype(
        np.float32
    )
    return a_blk, cvec, wvo_blk, bo2_rep


def _pack_slabs(x):
    """[B, N, D] -> per-batch [4*128, 1024] bf16 slab layout."""
    # [b, cblk, j, c4, s, f] -> [b, cblk, c4, f, j, s]
    y = np.asarray(x, np.float32).reshape(B, 4, GROUPS_PER_SC, G, S, D)
    y = y.transpose(0, 1, 3, 5, 2, 4).astype(NP_BF16)
    return y.reshape(B, 4 * 128, SLAB_COLS)


def make_in_maps(h_pos, h_geo, Wq, bq, Wk, bk, Wv, bv, Wo, bo):
    a_blk, cvec, wvo_blk, bo2_rep = _host_fold(Wq, bq, Wk, bk, Wv, bv, Wo, bo)
    hg_all = _pack_slabs(h_geo)
    hp_all = _pack_slabs(h_pos)
    in_maps = []
    for core in range(N_CORES):
        b0 = core * B_LOC
        in_maps.append(
            {
                "hg_p": np.ascontiguousarray(
                    hg_all[b0 : b0 + B_LOC].reshape(N_SC * 128, SLAB_COLS)
                ),
                "hp_p": np.ascontiguousarray(
                    hp_all[b0 : b0 + B_LOC].reshape(N_SC * 128, SLAB_COLS)
                ),
                "a_blk": a_blk,
                "cvec": cvec,
                "wvo_blk": wvo_blk,
                "bo2_rep": bo2_rep,
            }
        )
    return in_maps


def _unpack_out(out_dram):
    """[128, N_PAIRS*264] fp32 unnormalized -> [B_LOC, N, D] (host divide)."""
    y = out_dram.reshape(S, N_PAIRS, 2, G, 33)  # [s, pair, g, c, 33]
    f = y[..., :32]
    r = y[..., 32:33]
    o = f / r                                   # softmax normalization
    # pair,g -> group j (2 per pair); cluster = (sc, j, c)
    o = o.transpose(1, 2, 3, 0, 4)              # [pair, g, c, s, d]
    return o.reshape(B_LOC, N, D)


def kernel(h_pos, h_geo, n_clusters, Wq, bq, Wk, bk, Wv, bv, Wo, bo, **kwargs):
    assert int(n_clusters) == C_TOTAL
    nc = _get_program()
    in_maps = make_in_maps(h_pos, h_geo, Wq, bq, Wk, bk, Wv, bv, Wo, bo)
    res = run_bass_kernel_spmd(nc, in_maps, core_ids=list(range(N_CORES)))
    shards = [_unpack_out(r["out"]) for r in res.results]
    return np.concatenate(shards, axis=0).astype(np.float32)


# revision 7
# speedup vs baseline: 3.4649x; 1.0305x over previous
"""DLSA block (clustered sparse attention) Trainium2 kernel.

Full-input contract: kernel(**inputs) takes the complete unsharded tensors,
shards batch-dim across 8 NeuronCores, runs a Bass/Tile kernel per core, and
gathers the full output on host.

v3: bf16 datapath + software-pipelined pair loop.
  - All matmul operands bf16 (fp32 PSUM accumulate), rel err ~6e-3.
  - Scores for TWO groups share one 4-bank PSUM generation (8 row-banded
    matmuls; bank c holds clusters c of both groups at col offsets 0/128),
    so there is ONE exp ACTIVATE per 8 clusters and the scalar engine runs
    only exp (~33us). The exp->next-scores WAR dependency is covered by
    issuing V matmuls, the previous pair's F matmuls, and the Z matmuls of
    the next superchunk inside the window.
  - Normalization moved to host: F is stored UNNORMALIZED together with the
    softmax denominator column (33 cols per cluster); the unpack step does
    out = F/r. On-device vector work is only z-evac, V bias-add, F evac.
  - Host packs inputs into slab layout ([sc, (c4 f)=128, (j s)=1024] bf16,
    2KB DMA rows); output leaves as [s=128, (sc, pair, g, c, 33)] fp32.

Algebraic folds done on host (weight-space only, float64 for accuracy):
  A    = Wq^T @ Wk / sqrt(D)      -> scores S = Xg A Xg^T + (bq Wk/sqrt(D)) Xg^T
  bk drops entirely (softmax-invariant row constant).
  Wvo  = Wo @ Wv                  -> V' = Xp Wvo^T  (V and O projections fused)
  bo2  = bo + Wo @ bv             (bv commutes through attention; added to V')
"""

import sys

for _p in ("/opt/trn_rl_repo",):
    if _p not in sys.path:
        sys.path.insert(0, _p)

from contextlib import ExitStack

import numpy as np
import ml_dtypes

import concourse.bass as bass
import concourse.tile as tile
from concourse import bacc, mybir
from concourse.bass_utils import run_bass_kernel_spmd

F32 = mybir.dt.float32
BF16 = mybir.dt.bfloat16
NP_BF16 = ml_dtypes.bfloat16

B, N, D = 16, 16384, 32
C_TOTAL, S = 128, 128          # clusters per batch, points per cluster
N_CORES = 8
B_LOC = B // N_CORES           # batches per core
ROWS = B_LOC * N               # data rows per core
SC_CLUSTERS = 32               # clusters per superchunk
N_SC = ROWS // (SC_CLUSTERS * S)  # 8 superchunks per core
G = 4                          # clusters per group
GROUPS_PER_SC = SC_CLUSTERS // G  # 8
SLAB_COLS = GROUPS_PER_SC * S  # 1024 cols per input slab
PAIRS_PER_SC = GROUPS_PER_SC // 2  # 4
N_PAIRS = N_SC * PAIRS_PER_SC  # 32 pairs per core
FCOLS = 2 * G * 33             # 264 f-cols per pair (8 clusters x [32 vals | r])


def _build_program():
    nc = bacc.Bacc("TRN2", target_bir_lowering=False, debug=False)

    hg_p = nc.dram_tensor("hg_p", [N_SC * 128, SLAB_COLS], BF16,
                          kind="ExternalInput").ap()
    hp_p = nc.dram_tensor("hp_p", [N_SC * 128, SLAB_COLS], BF16,
                          kind="ExternalInput").ap()
    a_blk = nc.dram_tensor("a_blk", [128, 128], BF16, kind="ExternalInput").ap()
    cvec = nc.dram_tensor("cvec", [128, 1], F32, kind="ExternalInput").ap()
    wvo_blk = nc.dram_tensor("wvo_blk", [128, 128], BF16,
                             kind="ExternalInput").ap()
    bo2_rep = nc.dram_tensor("bo2_rep", [128, 2 * G * D], F32,
                             kind="ExternalInput").ap()
    out = nc.dram_tensor("out", [128, N_PAIRS * FCOLS], F32,
                         kind="ExternalOutput").ap()

    with tile.TileContext(nc) as tc, ExitStack() as ctx:
        consts = ctx.enter_context(tc.tile_pool(name="consts", bufs=1))
        io_pool = ctx.enter_context(tc.tile_pool(name="io", bufs=2))
        zsb_pool = ctx.enter_context(tc.tile_pool(name="zsb", bufs=2))
        pA_pool = ctx.enter_context(tc.tile_pool(name="pA", bufs=2))
        pB_pool = ctx.enter_context(tc.tile_pool(name="pB", bufs=2))
        out_pool = ctx.enter_context(tc.tile_pool(name="osc", bufs=2))
        v33_pool = ctx.enter_context(tc.tile_pool(name="v33", bufs=2))

        # PSUM: 8 banks of 2KB. Scores split into two 2-bank generations
        # (bands c0,c1 -> pool A; c2,c3 -> pool B) so the next pair's A-half
        # can run while expB of this pair still drains: A2 + B2 + z1 + v1 +
        # f2 = 8 banks.
        ps_z = ctx.enter_context(tc.tile_pool(name="ps_z", bufs=1, space="PSUM"))
        ps_wkA = ctx.enter_context(tc.tile_pool(name="ps_wkA", bufs=1, space="PSUM"))
        ps_wkB = ctx.enter_context(tc.tile_pool(name="ps_wkB", bufs=1, space="PSUM"))
        ps_v = ctx.enter_context(tc.tile_pool(name="ps_v", bufs=1, space="PSUM"))
        ps_f = ctx.enter_context(tc.tile_pool(name="ps_f", bufs=2, space="PSUM"))

        # constants
        a_sb = consts.tile([128, 128], BF16, tag="a_sb")
        nc.gpsimd.dma_start(a_sb[:], a_blk)
        cvec_sb = consts.tile([128, 1], F32, tag="cvec_sb")
        nc.gpsimd.dma_start(cvec_sb[:], cvec)
        wvo_sb = consts.tile([128, 128], BF16, tag="wvo_sb")
        nc.gpsimd.dma_start(wvo_sb[:], wvo_blk)
        bo2_sb = consts.tile([128, 2 * G * D], F32, tag="bo2_sb")
        nc.gpsimd.dma_start(bo2_sb[:], bo2_rep)

        # warm the exp table while the first slabs load
        warm = consts.tile([128, 1], F32, tag="warm")
        nc.scalar.activation(warm[:], cvec_sb[:],
                             mybir.ActivationFunctionType.Exp)

        # v33 ring: [t, (g c 33)] with ones in col 32 of each 33-block
        v33_tiles = []
        for i in range(2):
            t = v33_pool.tile([128, FCOLS], BF16, tag=f"v33_{i}")
            ones_ap = t[:].rearrange("p (c g) -> p c g", g=33)[:, :, 32:33]
            nc.gpsimd.memset(ones_ap, 1.0)
            v33_tiles.append(t)

        slabs = {}

        def load_slabs(sc):
            hg_sc = io_pool.tile([128, SLAB_COLS], BF16, tag="hg_sc")
            hp_sc = io_pool.tile([128, SLAB_COLS], BF16, tag="hp_sc")
            r0 = sc * 128
            nc.sync.dma_start(hg_sc[:], hg_p[r0 : r0 + 128, :])
            nc.sync.dma_start(hp_sc[:], hp_p[r0 : r0 + 128, :])
            slabs[sc] = (hg_sc, hp_sc)

        zsb = {}

        def z_phase(sc, h):
            """Z'^T half-slab via one wide matmul + DVE evac (bias cvec)."""
            hg_sc, _ = slabs[sc]
            if h == 0:
                zsb[sc] = zsb_pool.tile([128, SLAB_COLS], BF16, tag="z_sb", name="z_sb")
            half = slice(h * 512, (h + 1) * 512)
            z_ps = ps_z.tile([128, 512], F32, tag="z_ps")
            nc.tensor.matmul(z_ps[:], a_sb[:], hg_sc[:, half])
            nc.vector.tensor_scalar_add(zsb[sc][:, half], z_ps[:], cvec_sb[:])

        # state carried across pairs for the lag-1 F stage
        prev = None  # (p_sb2, v33, f2 slot filled later)
        f_tiles = {}
        out_tiles = {}

        def emit_F(P):
            """F matmuls for pair P (reads p_sb2/v33 of P), into f2[P]."""
            p_sbA, p_sbB, v33 = prev_state[P]
            f2 = ps_f.tile([128, FCOLS], F32, tag="f2", name="f2")
            f_tiles[P] = f2
            for g in range(2):
                for c in range(G):
                    col = (g * G + c) * 33
                    p_sb = p_sbA if c < 2 else p_sbB
                    ci = c % 2
                    nc.tensor.matmul(
                        f2[:, col : col + 33],
                        p_sb[:, ci * 256 + g * 128 : ci * 256 + (g + 1) * 128],
                        v33[:, col : col + 33],
                        tile_position=(0, 0),
                    )

        def emit_F_evac(P):
            """Copy f2[P] (unnormalized F + r cols) into the sc out tile."""
            sc = P // PAIRS_PER_SC
            p = P % PAIRS_PER_SC
            if p == 0:
                out_tiles[sc] = out_pool.tile([128, PAIRS_PER_SC * FCOLS],
                                              F32, tag="out_sc", name="out_sc")
            nc.vector.tensor_copy(
                out_tiles[sc][:, p * FCOLS : (p + 1) * FCOLS],
                f_tiles[P][:],
            )
            del f_tiles[P]
            if p % 2 == 1:  # store each completed half-sc immediately
                h0 = (p - 1) * FCOLS
                base = sc * PAIRS_PER_SC * FCOLS
                nc.gpsimd.dma_start(
                    out[:, base + h0 : base + h0 + 2 * FCOLS],
                    out_tiles[sc][:, h0 : h0 + 2 * FCOLS],
                )

        prev_state = {}

        # prologue
        load_slabs(0)
        z_phase(0, 0)
        z_phase(0, 1)

        for P in range(N_PAIRS):
            sc, p = P // PAIRS_PER_SC, P % PAIRS_PER_SC
            hg_sc, hp_sc = slabs[sc]
            z_sb = zsb[sc]
            if p == 0 and sc + 1 < N_SC:
                load_slabs(sc + 1)

            # scores: 8 row-banded matmuls in two 2-bank halves.
            # Half A = bands c0,c1; half B = bands c2,c3; within a half,
            # band i (cols i*512..) holds group g at +g*128.
            p_half = []
            for half_i, (ps_pool, p_pool, c_lo) in enumerate(
                ((ps_wkA, pA_pool, 0), (ps_wkB, pB_pool, 2))
            ):
                wk = ps_pool.tile([128, 1024], F32, tag="wk", name="wk")
                for g in range(2):
                    j = p * 2 + g
                    jcols = slice(j * S, (j + 1) * S)
                    for ci in range(2):
                        c = c_lo + ci
                        p0 = c * 32
                        nc.tensor.matmul(
                            wk[:, ci * 512 + g * 128 : ci * 512 + (g + 1) * 128],
                            hg_sc[p0 : p0 + 32, jcols],
                            z_sb[p0 : p0 + 32, jcols],
                            tile_position=(p0, 0),
                        )
                wk_view = wk[:].rearrange("p (c q) -> p c q", q=512)
                p_sb = p_pool.tile([128, 512], BF16, tag="p_sb", name="p_sb")
                nc.scalar.activation(
                    p_sb[:].rearrange("p (c q) -> p c q", q=256),
                    wk_view[:, :, 0:256],
                    mybir.ActivationFunctionType.Exp,
                )
                p_half.append(p_sb)

            # V' for both groups -> v_ps [128, (g, c, 32)]
            v_ps = ps_v.tile([128, 256], F32, tag="v_ps")
            for g in range(2):
                j = p * 2 + g
                nc.tensor.matmul(
                    v_ps[:, g * 128 : (g + 1) * 128],
                    hp_sc[:, (j * S) : (j + 1) * S],
                    wvo_sb[:],
                )

            # lag-1 F matmuls fill the exp->scores window
            if P > 0:
                emit_F(P - 1)

            # Z matmuls of the next superchunk also land in the window
            if p == 2 and sc + 1 < N_SC:
                z_phase(sc + 1, 0)
            elif p == 3 and sc + 1 < N_SC:
                z_phase(sc + 1, 1)

            # V'' = V' + bo2 -> bf16 into the v33 ring (ones col preserved)
            v33 = v33_tiles[P % 2]
            nc.vector.tensor_tensor(
                v33[:].rearrange("p (c g) -> p c g", g=33)[:, :, 0:32],
                v_ps[:].rearrange("p (c g) -> p c g", g=D),
                bo2_sb[:].rearrange("p (c g) -> p c g", g=D),
                mybir.AluOpType.add,
            )
            prev_state[P] = (p_half[0], p_half[1], v33)

            if P > 0:
                emit_F_evac(P - 1)
                del prev_state[P - 1]

        # epilogue
        emit_F(N_PAIRS - 1)
        emit_F_evac(N_PAIRS - 1)

    nc.compile()
    return nc


_PROGRAM = None


def _get_program():
    global _PROGRAM
    if _PROGRAM is None:
        _PROGRAM = _build_program()
    return _PROGRAM


def _host_fold(Wq, bq, Wk, bk, Wv, bv, Wo, bo):
    Wq64, Wk64 = np.asarray(Wq, np.float64), np.asarray(Wk, np.float64)
    Wv64, Wo64 = np.asarray(Wv, np.float64), np.asarray(Wo, np.float64)
    bq64, bv64, bo64 = (np.asarray(x, np.float64) for x in (bq, bv, bo))
    scale = 1.0 / np.sqrt(np.float64(D))
    A = (Wq64.T @ Wk64) * scale                      # [e, f]
    c = (bq64 @ Wk64) * scale                        # [f]
    WvoT = (Wo64 @ Wv64).T                           # [e, g]
    bo2 = bo64 + Wo64 @ bv64                         # [g]
    a_blk = np.zeros((128, 128), NP_BF16)
    wvo_blk = np.zeros((128, 128), NP_BF16)
    for cc in range(G):
        a_blk[cc * D : (cc + 1) * D, cc * D : (cc + 1) * D] = A.astype(NP_BF16)
        wvo_blk[cc * D : (cc + 1) * D, cc * D : (cc + 1) * D] = WvoT.astype(
            NP_BF16
        )
    cvec = np.tile(c, G)[:, None].astype(np.float32)         # [128, 1]
    bo2_rep = np.tile(bo2, (128, 2 * G)).reshape(128, 2 * G * D).astype(
        np.float32
    )
    return a_blk, cvec, wvo_blk, bo2_rep


def _pack_slabs(x):
    """[B, N, D] -> per-batch [4*128, 1024] bf16 slab layout."""
    # [b, cblk, j, c4, s, f] -> [b, cblk, c4, f, j, s]
    y = np.asarray(x, np.float32).reshape(B, 4, GROUPS_PER_SC, G, S, D)
    y = y.transpose(0, 1, 3, 5, 2, 4).astype(NP_BF16)
    return y.reshape(B, 4 * 128, SLAB_COLS)


def make_in_maps(h_pos, h_geo, Wq, bq, Wk, bk, Wv, bv, Wo, bo):
    a_blk, cvec, wvo_blk, bo2_rep = _host_fold(Wq, bq, Wk, bk, Wv, bv, Wo, bo)
    hg_all = _pack_slabs(h_geo)
    hp_all = _pack_slabs(h_pos)
    in_maps = []
    for core in range(N_CORES):
        b0 = core * B_LOC
        in_maps.append(
            {
                "hg_p": np.ascontiguousarray(
                    hg_all[b0 : b0 + B_LOC].reshape(N_SC * 128, SLAB_COLS)
                ),
                "hp_p": np.ascontiguousarray(
                    hp_all[b0 : b0 + B_LOC].reshape(N_SC * 128, SLAB_COLS)
                ),
                "a_blk": a_blk,
                "cvec": cvec,
                "wvo_blk": wvo_blk,
                "bo2_rep": bo2_rep,
            }
        )
    return in_maps


def _unpack_out(out_dram):
    """[128, N_PAIRS*264] fp32 unnormalized -> [B_LOC, N, D] (host divide)."""
    y = out_dram.reshape(S, N_PAIRS, 2, G, 33)  # [s, pair, g, c, 33]
    f = y[..., :32]
    r = y[..., 32:33]
    o = f / r                                   # softmax normalization
    # pair,g -> group j (2 per pair); cluster = (sc, j, c)
    o = o.transpose(1, 2, 3, 0, 4)              # [pair, g, c, s, d]
    return o.reshape(B_LOC, N, D)


def kernel(h_pos, h_geo, n_clusters, Wq, bq, Wk, bk, Wv, bv, Wo, bo, **kwargs):
    assert int(n_clusters) == C_TOTAL
    nc = _get_program()
    in_maps = make_in_maps(h_pos, h_geo, Wq, bq, Wk, bk, Wv, bv, Wo, bo)
    res = run_bass_kernel_spmd(nc, in_maps, core_ids=list(range(N_CORES)))
    shards = [_unpack_out(r["out"]) for r in res.results]
    return np.concatenate(shards, axis=0).astype(np.float32)
